# revision 45
# baseline (speedup 1.0000x reference)
# Trainium2 Bass kernel for nn_CrossAttention (8-core SPMD).
#
# Reference computation (fp32):
#   q = x @ Wq; k = ctx @ Wk; v = ctx @ Wv        (per-head d=64, 8 heads)
#   out = softmax(q k^T / sqrt(d)) v              (full attention)
#   y = out @ Wo + bo
#
# Sharding: 8 cores = 4 batches x 2 query-row halves. Each core gets one
# batch's context and half of that batch's 4096 query rows (2048 rows), all
# weights, and produces its full [2048, 1024] output slice independently —
# no collectives, host only concatenates.
#
# Steady state is co-limited by the PE matmul stream (~41 512-col slots per
# (query-chunk, head-pair) body) and the ScalarE exp stream (8 x ~1.1us per
# body), which are balanced to within a few %. The schedule (vs the naive
# phase-by-phase version, 223.6us -> ~210.4us):
#   - emits each body's 8 score j-slots in PAIRS, each pair followed by a
#     slot-weighted slice of the body's "filler" matmuls (previous pair's
#     PV, next chunk's q projection, an output-projection slice, setup
#     work). Pairing needs the 3-deep score-PSUM ring (sc tag, 6 banks;
#     acc tag 2 banks holds everything else) and keeps filler blocks
#     large: every accumulation-group boundary in the PE stream costs
#     ~100ns of exposed LDWEIGHTS, so fewer/larger blocks win;
#   - starts the pipeline after a minimal prefix (6 kT + 8 qT matmuls on a
#     criticality-ordered DMA stream; Wq/Wk arrive host-packed ic-major so
#     the ic0 slices are single contiguous transfers), deferring the
#     remaining ~110 setup matmuls into the first bodies' fillers; two
#     blocks of dummy matmuls over a memset tile burn the DMA waits so the
#     HAM clock-gate keeps the PE at 2.4 GHz throughout;
#   - broadcasts each pair's softmax denominators with ONE K=33 matmul
#     (den rows on partitions 0 and 32 — both legal 32-aligned starts —
#     with memset zeros between) instead of two K=1 matmuls, and places
#     fin last in each body's fillers; output-projection groups run one
#     body later than their attn chunk's last normalize (yo map shift) so
#     nothing reads attn before its fin;
#   - splits the LAST chunk's output projection partial-K: ic0-2
#     pre-accumulate into the score-ring banks freed by the final exps
#     while fin's DVE chain runs; only 8 ic3 finishing matmuls + bias adds
#     (4-deep y ring) remain on the post-fin tail;
#   - writes the output in bf16 (halves the output DMA; ~0.1% extra error
#     against a 2e-2 budget).
import os

import numpy as np

import concourse.bass as bass
import concourse.tile as tile
from concourse import bacc, mybir
from concourse.bass_utils import run_bass_kernel_spmd

F32 = mybir.dt.float32
BF16 = mybir.dt.bfloat16
EXP = mybir.ActivationFunctionType.Exp
P = 128

B = 4
SQ_FULL = 4096
SQ = 2048          # per-core query rows
SKV = 1024
DQ = 1024
DKV = 768
INNER = 512
H = 8
DH = 64
SQC = 512          # query-chunk (matmul free dim)
NSQ = SQ // SQC    # 4
KCQ = DQ // P      # 8
KCK = DKV // P     # 6
NIC = INNER // P   # 4 inner-dim partition chunks
NJ = SKV // P      # 8 key chunks
NHP = H // 2       # 4 head pairs
SCALE = 1.0 / 8.0  # dh ** -0.5


def _mm(nc, out, lhsT, rhs, start, stop):
    nc.tensor.matmul(out, lhsT, rhs, start=start, stop=stop)


def build_nc():
    nc = bacc.Bacc(trn_type="TRN2", target_bir_lowering=False, debug=False)
    xT = nc.dram_tensor("xT", [DQ, SQ], BF16, kind="ExternalInput").ap()
    ctxT = nc.dram_tensor("ctxT", [DKV, SKV], BF16, kind="ExternalInput").ap()
    # Wq/Wk arrive host-packed ic-major ([ic][p][kc][n]) so the ic0 slice
    # the prefix needs is one small contiguous DMA with >=1.5KB lines —
    # column-slicing the natural layout costs ~2.5us of descriptor
    # generation on the serial Sync queue and transfers at 256B lines.
    Wq = nc.dram_tensor("Wq", [NIC, P, KCQ, P], BF16, kind="ExternalInput").ap()
    Wk = nc.dram_tensor("Wk", [NIC, P, KCK, P], BF16, kind="ExternalInput").ap()
    Wv = nc.dram_tensor("Wv", [DKV, INNER], BF16, kind="ExternalInput").ap()
    Wo = nc.dram_tensor("Wo", [INNER, DQ], BF16, kind="ExternalInput").ap()
    bo = nc.dram_tensor("bo", [DQ], F32, kind="ExternalInput").ap()
    selm = nc.dram_tensor("selm", [33, P], BF16, kind="ExternalInput").ap()
    y = nc.dram_tensor("y", [SQ, DQ], BF16, kind="ExternalOutput").ap()

    with tile.TileContext(nc) as tc:
        with (
            tc.tile_pool(name="res", bufs=1) as res,
            tc.tile_pool(name="setup", bufs=1) as setup,
            tc.tile_pool(name="xq", bufs=2) as xq,
            tc.tile_pool(name="qt", bufs=2) as qt,
            tc.tile_pool(name="ex", bufs=2) as ex,
            tc.tile_pool(name="at", bufs=2) as at,
            tc.tile_pool(name="yp", bufs=2) as yp,
            tc.tile_pool(name="rc", bufs=2) as rc,
            tc.tile_pool(name="ps", bufs=2, space="PSUM") as psp,
        ):
            # --- input DMAs, pipeline-critical-path order: the prefix
            # (kT ic0/nk0 + qT s0/ic0) needs ctx half 0, the ic0 column
            # slices of Wk/Wq, and x0 — the remaining weight columns
            # stream in behind them.
            ctx_sb = setup.tile([P, KCK, SKV], BF16)
            nc.sync.dma_start(
                ctx_sb[:, :, 0:512],
                ctxT[:, 0:512].rearrange("(c p) m -> p c m", p=P))
            Wk_sb = setup.tile([P, NIC, KCK, P], BF16)
            nc.sync.dma_start(Wk_sb[:, 0], Wk[0])

            xT_tiles = {}

            def load_xT(s):
                t = xq.tile([P, KCQ, SQC], BF16, name=f"xT{s}", tag="xT")
                nc.sync.dma_start(
                    t[:],
                    xT[:, s * SQC:(s + 1) * SQC]
                    .rearrange("(c p) m -> p c m", p=P),
                )
                xT_tiles[s] = t

            load_xT(0)
            Wq_sb = res.tile([P, NIC, KCQ, P], BF16)
            nc.sync.dma_start(Wq_sb[:, 0], Wq[0])
            nc.sync.dma_start(
                Wk_sb[:, 1:NIC],
                Wk[1:NIC].rearrange("i p c n -> p i c n"))
            nc.sync.dma_start(
                Wq_sb[:, 1:NIC],
                Wq[1:NIC].rearrange("i p c n -> p i c n"))
            nc.sync.dma_start(
                ctx_sb[:, :, 512:1024],
                ctxT[:, 512:1024].rearrange("(c p) m -> p c m", p=P))
            Wv_sb = setup.tile([P, KCK, INNER], BF16)
            nc.sync.dma_start(Wv_sb[:], Wv.rearrange("(c p) m -> p c m", p=P))
            Wo_sb = res.tile([P, NIC, DQ], BF16)
            nc.sync.dma_start(Wo_sb[:], Wo.rearrange("(c p) m -> p c m", p=P))
            bo_sb = res.tile([P, DQ], F32)
            nc.sync.dma_start(bo_sb[:], bo.unsqueeze(0).broadcast_to([P, DQ]))
            selm_sb = res.tile([33, P], BF16)
            nc.sync.dma_start(selm_sb[:], selm)
            load_xT(1)

            kT_sb = res.tile([P, NIC, SKV], BF16)
            v_sb = res.tile([P, NJ, H, DH + 1], BF16)
            ones_sb = res.tile([P, NJ * H], BF16)
            nc.vector.memset(ones_sb[:], 1.0)
            nc.vector.tensor_copy(
                v_sb[:, :, :, DH:DH + 1],
                ones_sb.rearrange("p (a b u) -> p a b u", a=NJ, u=1),
            )

            # --- matmul group emitters -----------------------------------
            def kT_group(ic, nk):
                # kT[ic*128:+128, nk*512:+512] = Wk[:, ic-chunk].T @ ctxT[:, nk-chunk]
                ps = psp.tile([P, 512], F32, tag="acc", name=f"kp{ic}_{nk}")
                for kc in range(KCK):
                    _mm(nc, ps[:], Wk_sb[:, ic, kc, :],
                        ctx_sb[:, kc, nk * 512:(nk + 1) * 512],
                        kc == 0, kc == KCK - 1)
                nc.vector.tensor_copy(kT_sb[:, ic, nk * 512:(nk + 1) * 512], ps[:])

            qT_tiles = {}

            def qT_group(s, ic):
                # 8 matmuls projecting one inner-chunk of q for chunk s
                if ic == 0:
                    qT_tiles[s] = qt.tile([P, NIC, SQC], BF16, name=f"qT{s}", tag="qT")
                xT_sb = xT_tiles[s]
                ps = psp.tile([P, SQC], F32, tag="acc", name=f"qp{s}_{ic}")
                for kc in range(KCQ):
                    _mm(nc, ps[:], Wq_sb[:, ic, kc, :],
                        xT_sb[:, kc, :], kc == 0, kc == KCQ - 1)
                nc.vector.tensor_copy(qT_tiles[s][:, ic, :], ps[:])

            def v_group(j):
                # v natural: v[j*128:+128, :] = ctxT[:, j-chunk].T @ Wv
                ps = psp.tile([P, INNER], F32, tag="acc", name=f"vp{j}")
                for kc in range(KCK):
                    _mm(nc, ps[:], ctx_sb[:, kc, j * P:(j + 1) * P],
                        Wv_sb[:, kc, :], kc == 0, kc == KCK - 1)
                nc.vector.tensor_copy(
                    v_sb[:, j, :, 0:DH],
                    ps.rearrange("p (h d) -> p h d", h=H),
                )

            attn_tiles = {}

            def make_pv(ps_, hp_, expb_):
                # previous pair's PV accumulation; row dh of each head's
                # block is the softmax denominator (ones column of v).
                st = {}

                def part(par, j):
                    if j == 0:
                        st[par] = psp.tile([P, SQC], F32, tag="acc",
                                           name=f"pv{ps_}_{2 * hp_ + par}")
                    pv = st[par]
                    _mm(nc, pv[0:DH + 1, :], v_sb[:, j, 2 * hp_ + par, :],
                        expb_[:, j, par, :], j == 0, j == NJ - 1)
                    if j == NJ - 1:
                        po = par * DH
                        nc.vector.tensor_copy(
                            attn_tiles[ps_][po:po + DH, hp_, :], pv[0:DH, :])
                        if par == 0:
                            # den rows live on partitions 0 and 32 (both
                            # 32-aligned starts) with zeros between, so one
                            # K=33 matmul broadcasts both heads' denominators
                            st["den"] = rc.tile([33, SQC], BF16, tag="den",
                                                name=f"dn{ps_}_{hp_}")
                            nc.vector.memset(st["den"][:], 0.0)
                        nc.vector.tensor_copy(
                            st["den"][32 * par:32 * par + 1, :],
                            pv[DH:DH + 1, :])

                def fin():
                    # broadcast den over each head's 64-partition block (one
                    # K=33 matmul; selm row 0 has ones on partitions 0..63,
                    # row 32 on 64..127), then reciprocal across all 128
                    # lanes and normalize that inner-chunk of attn
                    den = st["den"]
                    rps = psp.tile([P, SQC], F32, tag="acc",
                                   name=f"rb{ps_}_{hp_}")
                    _mm(nc, rps[:], selm_sb[:], den[:], True, True)
                    rrec = rc.tile([P, SQC], F32, tag="rrec",
                                   name=f"rr{ps_}_{hp_}")
                    nc.vector.reciprocal_approx_fast(rrec[:], rps[:])
                    nc.vector.tensor_mul(
                        attn_tiles[ps_][:, hp_, :],
                        attn_tiles[ps_][:, hp_, :], rrec[:])

                return part, fin

            def yo_group(s, r):
                # output projection + bias for one 128-row slice of chunk s
                y_sb = yp.tile([P, DQ], BF16, tag="y", name=f"y{s}_{r}")
                for nh in range(DQ // 512):
                    ps = psp.tile([P, 512], F32, tag="acc", name=f"yp{s}_{r}_{nh}")
                    for kc in range(NIC):
                        _mm(nc, ps[:],
                            attn_tiles[s][:, kc, r * P:(r + 1) * P],
                            Wo_sb[:, kc, nh * 512:(nh + 1) * 512],
                            kc == 0, kc == NIC - 1)
                    nc.vector.tensor_add(
                        y_sb[:, nh * 512:(nh + 1) * 512], ps[:],
                        bo_sb[:, nh * 512:(nh + 1) * 512])
                nc.sync.dma_start(
                    y[s * SQC + r * P: s * SQC + (r + 1) * P, :], y_sb[:])

            # --- HAM warm-up: the PE clock-gates to 1.2 GHz until it has
            # been busy ~3.4us. Burn the input-DMA wait on dummy matmuls
            # over a memset tile so the prefix and first body run at
            # 2.4 GHz instead of paying ~2x on ~14 matmuls.
            warm_sb = res.tile([P, 640], BF16)
            nc.vector.memset(warm_sb[:], 0.125)
            for w in range(12):
                wps = psp.tile([P, 512], F32, tag="acc", name=f"warm{w}")
                _mm(nc, wps[:], warm_sb[:, 0:P], warm_sb[:, P:640],
                    True, True)

            # --- minimal prefix: exactly what body (0,0) scores j0..j3
            # need (kT ic0/nk0 + qT s0/ic0); everything else is filler.
            # A second dummy block bridges the x0+Wq0 DMA wait so the HAM
            # stays warm for qT and the first body.
            kT_group(0, 0)
            for w in range(12, 24):
                wps = psp.tile([P, 512], F32, tag="acc", name=f"warm{w}")
                _mm(nc, wps[:], warm_sb[:, 0:P], warm_sb[:, P:640],
                    True, True)
            qT_group(0, 0)

            # --- filler machinery: (slot_weight, closure) queue, drained
            # in slot-weighted slices after each score of each body.
            fill = []

            def add(w, fn):
                fill.append((w, fn))

            def drain(target_w):
                done = 0.0
                while fill and done < target_w:
                    w, fn = fill.pop(0)
                    fn()
                    done += w

            def add_pv(part):
                for par in range(2):
                    for j in range(NJ):
                        add(1, lambda par=par, j=j: part(par, j))

            pend = None  # (part, fin) of the previous head pair

            # --- main pipeline over (query chunk, head pair) bodies ------
            for s in range(NSQ):
                attn_tiles[s] = at.tile([P, NIC, SQC], BF16, name=f"at{s}", tag="at")
                if s == 0:
                    pass
                elif s == 1:
                    load_xT(2)
                elif s == 2:
                    load_xT(3)
                for hp in range(NHP):
                    # build this body's filler list. fin goes LAST: with it
                    # early, the next three acc-ring allocations (qT, yo)
                    # all chain on fin's DVE latency (den copy -> rps ->
                    # reciprocal) and the PE stalls ~0.5-1us every body
                    # boundary.
                    if pend is not None:
                        part_prev, fin_prev = pend
                        add_pv(part_prev)
                        if (s, hp) == (NSQ - 1, NHP - 1):
                            # last body: fin mid-body so the epilogue's
                            # acc-ring allocations don't chain on its DVE
                            # latency at the critical tail.
                            add(1, fin_prev)
                            fin_prev = None
                    else:
                        fin_prev = None
                    if s == 0:
                        if hp == 0:
                            add(6, lambda: kT_group(0, 1))
                            add(6, lambda: kT_group(1, 0))
                            add(6, lambda: kT_group(1, 1))
                            for j in range(NJ):
                                add(6, lambda j=j: v_group(j))
                            add(8, lambda: qT_group(0, 1))
                        elif hp == 1:
                            add(6, lambda: kT_group(2, 0))
                            add(6, lambda: kT_group(2, 1))
                            add(8, lambda: qT_group(0, 2))
                        elif hp == 2:
                            add(6, lambda: kT_group(3, 0))
                            add(6, lambda: kT_group(3, 1))
                            add(8, lambda: qT_group(0, 3))
                            add(8, lambda: qT_group(1, 0))
                        else:
                            for ic in range(1, NIC):
                                add(8, lambda ic=ic: qT_group(1, ic))
                    else:
                        # yo(sp, r) must trail fin(sp, 3) (which finalizes
                        # attn[sp]) by at least one body, so the map is
                        # shifted: body (s,hp>=1) runs yo(s-1, hp-1) and
                        # body (s,0) runs yo(s-2, 3). yo(2,3) + yo(3,*)
                        # land in the epilogue.
                        if s + 1 < NSQ:
                            add(8, lambda s=s, hp=hp: qT_group(s + 1, hp))
                        if hp == 0:
                            if s >= 2:
                                add(8, lambda s=s: yo_group(s - 2, 3))
                        else:
                            add(8, lambda s=s, hp=hp: yo_group(s - 1, hp - 1))
                        if (s, hp) == (NSQ - 1, NHP - 1):
                            add(8, lambda: yo_group(2, 3))
                    if fin_prev is not None:
                        add(1, fin_prev)

                    total_w = sum(w for w, _ in fill)
                    qT_sb = qT_tiles[s]
                    expb = ex.tile([P, NJ, 2, SQC], BF16, tag="expb",
                                   name=f"eb{s}_{hp}")
                    emitted = 0.0
                    for j in range(NJ):
                        # scores^T [Skv, SQC] for the head pair; the two K=64
                        # matmuls land on disjoint PE row groups, share one
                        # 2-bank PSUM tile and one exp call, so the A/B row
                        # groups execute concurrently. Scores are emitted in
                        # j-PAIRS (the 3-deep sc ring gives the second score
                        # the slack) so the filler blocks between them stay
                        # large — every group boundary in the PE stream costs
                        # ~100ns of exposed LDWEIGHTS.
                        sps = psp.tile([P, 2, 512], F32, tag="sc", bufs=3,
                                       name=f"sc{s}_{hp}_{j}")
                        _mm(nc, sps[:, 0, :],
                            kT_sb[0:DH, hp, j * P:(j + 1) * P],
                            qT_sb[0:DH, hp, :], True, True)
                        _mm(nc, sps[:, 1, :],
                            kT_sb[DH:P, hp, j * P:(j + 1) * P],
                            qT_sb[DH:P, hp, :], True, True)
                        nc.scalar.activation(
                            expb[:, j, :, :], sps[:], EXP, scale=SCALE)
                        if j % 2 == 1:
                            quota = total_w * (j + 1) / NJ - emitted
                            drain(quota)
                            emitted += quota
                    pend = make_pv(s, hp, expb)

            # --- epilogue: drain leftovers, last pair's PV + fin. The last
            # chunk's output projection is split partial-K: the ic0..2
            # contributions (normalized several bodies ago) pre-accumulate
            # into the score-ring banks (free once the last exps retire)
            # while fin's DVE chain runs, and only the 8 ic3 finishing
            # matmuls remain on the post-fin critical path.
            drain(1e9)
            part_last, fin_last = pend
            # j0-5 of both pars first: the exp-j6/j7 waits then sit ~12
            # slots later in the queue and never stall the PE.
            for par in range(2):
                for j in range(NJ - 2):
                    part_last(par, j)
            for j in range(NJ - 2, NJ):
                part_last(0, j)
                part_last(1, j)
            fin_last()
            sl = NSQ - 1
            ypp = {}
            for r in range(3):
                t = psp.tile([P, 2, 512], F32, tag="sc", bufs=3,
                             name=f"ypp{r}")
                for nh in range(2):
                    ypp[(r, nh)] = t[:, nh, :]
                    for kc in range(NIC - 1):
                        _mm(nc, t[:, nh, :],
                            attn_tiles[sl][:, kc, r * P:(r + 1) * P],
                            Wo_sb[:, kc, nh * 512:(nh + 1) * 512],
                            kc == 0, False)
            for nh in range(2):
                t = psp.tile([P, 512], F32, tag="acc", name=f"ypp3_{nh}")
                ypp[(3, nh)] = t[:]
                for kc in range(NIC - 1):
                    _mm(nc, t[:],
                        attn_tiles[sl][:, kc, 3 * P:4 * P],
                        Wo_sb[:, kc, nh * 512:(nh + 1) * 512],
                        kc == 0, False)
            for r in range(NHP):
                # 4-deep ring: with bufs=2 the third slice's bias-add waits
                # the first slice's output DMA and the tail serializes.
                y_sb = yp.tile([P, DQ], BF16, tag="yf", bufs=4, name=f"yf{r}")
                for nh in range(2):
                    _mm(nc, ypp[(r, nh)],
                        attn_tiles[sl][:, NIC - 1, r * P:(r + 1) * P],
                        Wo_sb[:, NIC - 1, nh * 512:(nh + 1) * 512],
                        False, True)
                    nc.vector.tensor_add(
                        y_sb[:, nh * 512:(nh + 1) * 512], ypp[(r, nh)],
                        bo_sb[:, nh * 512:(nh + 1) * 512])
                nc.sync.dma_start(
                    y[sl * SQC + r * P: sl * SQC + (r + 1) * P, :], y_sb[:])
    nc.compile()
    return nc


_NC_CACHE = None


def kernel(x, context, Wq, Wk, Wv, Wo, bo):
    global _NC_CACHE
    import ml_dtypes
    bf16 = ml_dtypes.bfloat16

    x = np.asarray(x, dtype=np.float32)
    context = np.asarray(context, dtype=np.float32)
    # Wq/Wk packed ic-major: [ic][p][kc][n] with element (ic,p,kc,n) =
    # W[kc*128+p, ic*128+n], so the kernel's prefix can DMA just the ic0
    # slice contiguously.
    Wq_b = np.asarray(Wq, dtype=np.float32).astype(bf16) \
        .reshape(KCQ, P, NIC, P).transpose(2, 1, 0, 3)
    Wq_b = np.ascontiguousarray(Wq_b)
    Wk_b = np.asarray(Wk, dtype=np.float32).astype(bf16) \
        .reshape(KCK, P, NIC, P).transpose(2, 1, 0, 3)
    Wk_b = np.ascontiguousarray(Wk_b)
    Wv_b = np.ascontiguousarray(np.asarray(Wv, dtype=np.float32).astype(bf16))
    Wo_b = np.ascontiguousarray(np.asarray(Wo, dtype=np.float32).astype(bf16))
    bo = np.ascontiguousarray(np.asarray(bo, dtype=np.float32))

    if _NC_CACHE is None:
        _NC_CACHE = build_nc()
    nc = _NC_CACHE

    selm = np.zeros((33, P), dtype=np.float32)
    selm[0, 0:DH] = 1.0
    selm[32, DH:2 * DH] = 1.0
    selm = np.ascontiguousarray(selm.astype(bf16))

    in_maps = []
    for c in range(8):
        b, half = c // 2, c % 2
        xs = x[b, half * SQ:(half + 1) * SQ, :]            # [2048, 1024]
        in_maps.append({
            "xT": np.ascontiguousarray(xs.T.astype(bf16)),       # [1024, 2048]
            "ctxT": np.ascontiguousarray(context[b].T.astype(bf16)),  # [768, 1024]
            "Wq": Wq_b, "Wk": Wk_b, "Wv": Wv_b, "Wo": Wo_b, "bo": bo,
            "selm": selm,
        })

    trace = bool(int(os.environ.get("KERNEL_TRACE", "0")))
    res = run_bass_kernel_spmd(nc, in_maps, core_ids=list(range(8)), trace=trace)
    kernel.last_results = res

    out = np.empty((B, SQ_FULL, DQ), dtype=np.float32)
    for c in range(8):
        b, half = c // 2, c % 2
        out[b, half * SQ:(half + 1) * SQ, :] = \
            res.results[c]["y"].astype(np.float32)
    return out


# revision 46
# speedup vs baseline: 1.0098x; 1.0098x over previous
# Trainium2 Bass kernel for nn_CrossAttention (8-core SPMD).
#
# Reference computation (fp32):
#   q = x @ Wq; k = ctx @ Wk; v = ctx @ Wv        (per-head d=64, 8 heads)
#   out = softmax(q k^T / sqrt(d)) v              (full attention)
#   y = out @ Wo + bo
#
# Sharding: 8 cores = 4 batches x 2 query-row halves. Each core gets one
# batch's context and half of that batch's 4096 query rows (2048 rows), all
# weights, and produces its full [2048, 1024] output slice independently —
# no collectives, host only concatenates.
#
# Steady state is co-limited by the PE matmul stream (~41 512-col slots per
# (query-chunk, head-pair) body) and the ScalarE exp stream (8 x ~1.1us per
# body), which are balanced to within a few %. The schedule (vs the naive
# phase-by-phase version, 223.6us -> ~210.4us):
#   - emits each body's 8 score j-slots in PAIRS, each pair followed by a
#     slot-weighted slice of the body's "filler" matmuls (previous pair's
#     PV, next chunk's q projection, an output-projection slice, setup
#     work). Pairing needs the 3-deep score-PSUM ring (sc tag, 6 banks;
#     acc tag 2 banks holds everything else) and keeps filler blocks
#     large: every accumulation-group boundary in the PE stream costs
#     ~100ns of exposed LDWEIGHTS, so fewer/larger blocks win;
#   - starts the pipeline after a minimal prefix (6 kT + 8 qT matmuls on a
#     criticality-ordered DMA stream; Wq/Wk arrive host-packed ic-major so
#     the ic0 slices are single contiguous transfers), deferring the
#     remaining ~110 setup matmuls into the first bodies' fillers; two
#     blocks of dummy matmuls over a memset tile burn the DMA waits so the
#     HAM clock-gate keeps the PE at 2.4 GHz throughout;
#   - broadcasts each pair's softmax denominators with ONE K=33 matmul
#     (den rows on partitions 0 and 32 — both legal 32-aligned starts —
#     with memset zeros between) instead of two K=1 matmuls, and places
#     fin last in each body's fillers; output-projection groups run one
#     body later than their attn chunk's last normalize (yo map shift) so
#     nothing reads attn before its fin;
#   - splits the LAST chunk's output projection partial-K: ic0-2
#     pre-accumulate into the score-ring banks freed by the final exps
#     while fin's DVE chain runs; only 8 ic3 finishing matmuls + bias adds
#     (4-deep y ring) remain on the post-fin tail;
#   - writes the output in bf16 (halves the output DMA; ~0.1% extra error
#     against a 2e-2 budget).
import os

import numpy as np

import concourse.bass as bass
import concourse.tile as tile
from concourse import bacc, mybir
from concourse.bass_utils import run_bass_kernel_spmd

F32 = mybir.dt.float32
BF16 = mybir.dt.bfloat16
EXP = mybir.ActivationFunctionType.Exp
P = 128

B = 4
SQ_FULL = 4096
SQ = 2048          # per-core query rows
SKV = 1024
DQ = 1024
DKV = 768
INNER = 512
H = 8
DH = 64
SQC = 512          # query-chunk (matmul free dim)
NSQ = SQ // SQC    # 4
KCQ = DQ // P      # 8
KCK = DKV // P     # 6
NIC = INNER // P   # 4 inner-dim partition chunks
NJ = SKV // P      # 8 key chunks
NHP = H // 2       # 4 head pairs
SCALE = 1.0 / 8.0  # dh ** -0.5


def _mm(nc, out, lhsT, rhs, start, stop):
    nc.tensor.matmul(out, lhsT, rhs, start=start, stop=stop)


def build_nc():
    nc = bacc.Bacc(trn_type="TRN2", target_bir_lowering=False, debug=False)
    xT = nc.dram_tensor("xT", [DQ, SQ], BF16, kind="ExternalInput").ap()
    ctxT = nc.dram_tensor("ctxT", [DKV, SKV], BF16, kind="ExternalInput").ap()
    # Wq/Wk arrive host-packed ic-major ([ic][p][kc][n]) so the ic0 slice
    # the prefix needs is one small contiguous DMA with >=1.5KB lines —
    # column-slicing the natural layout costs ~2.5us of descriptor
    # generation on the serial Sync queue and transfers at 256B lines.
    Wq = nc.dram_tensor("Wq", [NIC, P, KCQ, P], BF16, kind="ExternalInput").ap()
    Wk = nc.dram_tensor("Wk", [NIC, P, KCK, P], BF16, kind="ExternalInput").ap()
    Wv = nc.dram_tensor("Wv", [DKV, INNER], BF16, kind="ExternalInput").ap()
    Wo = nc.dram_tensor("Wo", [INNER, DQ], BF16, kind="ExternalInput").ap()
    bo = nc.dram_tensor("bo", [DQ], F32, kind="ExternalInput").ap()
    selm = nc.dram_tensor("selm", [33, P], BF16, kind="ExternalInput").ap()
    y = nc.dram_tensor("y", [SQ, DQ], BF16, kind="ExternalOutput").ap()

    with tile.TileContext(nc) as tc:
        with (
            tc.tile_pool(name="res", bufs=1) as res,
            tc.tile_pool(name="setup", bufs=1) as setup,
            tc.tile_pool(name="xq", bufs=2) as xq,
            tc.tile_pool(name="qt", bufs=2) as qt,
            tc.tile_pool(name="ex", bufs=2) as ex,
            tc.tile_pool(name="at", bufs=2) as at,
            tc.tile_pool(name="yp", bufs=2) as yp,
            tc.tile_pool(name="rc", bufs=2) as rc,
            tc.tile_pool(name="ps", bufs=2, space="PSUM") as psp,
        ):
            # --- input DMAs, pipeline-critical-path order: the prefix
            # (kT ic0/nk0 + qT s0/ic0) needs ctx half 0, the ic0 column
            # slices of Wk/Wq, and x0 — the remaining weight columns
            # stream in behind them.
            ctx_sb = setup.tile([P, KCK, SKV], BF16)
            nc.sync.dma_start(
                ctx_sb[:, :, 0:512],
                ctxT[:, 0:512].rearrange("(c p) m -> p c m", p=P))
            Wk_sb = setup.tile([P, NIC, KCK, P], BF16)
            nc.sync.dma_start(Wk_sb[:, 0], Wk[0])

            xT_tiles = {}

            def load_xT(s):
                t = xq.tile([P, KCQ, SQC], BF16, name=f"xT{s}", tag="xT")
                nc.sync.dma_start(
                    t[:],
                    xT[:, s * SQC:(s + 1) * SQC]
                    .rearrange("(c p) m -> p c m", p=P),
                )
                xT_tiles[s] = t

            load_xT(0)
            Wq_sb = res.tile([P, NIC, KCQ, P], BF16)
            nc.sync.dma_start(Wq_sb[:, 0], Wq[0])
            nc.sync.dma_start(
                Wk_sb[:, 1:NIC],
                Wk[1:NIC].rearrange("i p c n -> p i c n"))
            nc.sync.dma_start(
                Wq_sb[:, 1:NIC],
                Wq[1:NIC].rearrange("i p c n -> p i c n"))
            nc.sync.dma_start(
                ctx_sb[:, :, 512:1024],
                ctxT[:, 512:1024].rearrange("(c p) m -> p c m", p=P))
            Wv_sb = setup.tile([P, KCK, INNER], BF16)
            nc.sync.dma_start(Wv_sb[:], Wv.rearrange("(c p) m -> p c m", p=P))
            Wo_sb = res.tile([P, NIC, DQ], BF16)
            nc.sync.dma_start(Wo_sb[:], Wo.rearrange("(c p) m -> p c m", p=P))
            bo_sb = res.tile([P, DQ], F32)
            nc.sync.dma_start(bo_sb[:], bo.unsqueeze(0).broadcast_to([P, DQ]))
            selm_sb = res.tile([33, P], BF16)
            nc.sync.dma_start(selm_sb[:], selm)
            load_xT(1)

            kT_sb = res.tile([P, NIC, SKV], BF16)
            v_sb = res.tile([P, NJ, H, DH + 1], BF16)
            ones_sb = res.tile([P, NJ * H], BF16)
            nc.vector.memset(ones_sb[:], 1.0)
            nc.vector.tensor_copy(
                v_sb[:, :, :, DH:DH + 1],
                ones_sb.rearrange("p (a b u) -> p a b u", a=NJ, u=1),
            )

            # --- matmul group emitters -----------------------------------
            def kT_group(ic, nk):
                # kT[ic*128:+128, nk*512:+512] = Wk[:, ic-chunk].T @ ctxT[:, nk-chunk]
                ps = psp.tile([P, 512], F32, tag="acc", name=f"kp{ic}_{nk}")
                for kc in range(KCK):
                    _mm(nc, ps[:], Wk_sb[:, ic, kc, :],
                        ctx_sb[:, kc, nk * 512:(nk + 1) * 512],
                        kc == 0, kc == KCK - 1)
                nc.vector.tensor_copy(kT_sb[:, ic, nk * 512:(nk + 1) * 512], ps[:])

            qT_tiles = {}

            def qT_group(s, ic):
                # 8 matmuls projecting one inner-chunk of q for chunk s
                if ic == 0:
                    qT_tiles[s] = qt.tile([P, NIC, SQC], BF16, name=f"qT{s}", tag="qT")
                xT_sb = xT_tiles[s]
                ps = psp.tile([P, SQC], F32, tag="acc", name=f"qp{s}_{ic}")
                for kc in range(KCQ):
                    _mm(nc, ps[:], Wq_sb[:, ic, kc, :],
                        xT_sb[:, kc, :], kc == 0, kc == KCQ - 1)
                nc.vector.tensor_copy(qT_tiles[s][:, ic, :], ps[:])

            def v_group(j):
                # v natural: v[j*128:+128, :] = ctxT[:, j-chunk].T @ Wv
                ps = psp.tile([P, INNER], F32, tag="acc", name=f"vp{j}")
                for kc in range(KCK):
                    _mm(nc, ps[:], ctx_sb[:, kc, j * P:(j + 1) * P],
                        Wv_sb[:, kc, :], kc == 0, kc == KCK - 1)
                nc.vector.tensor_copy(
                    v_sb[:, j, :, 0:DH],
                    ps.rearrange("p (h d) -> p h d", h=H),
                )

            attn_tiles = {}

            def make_pv(ps_, hp_, expb_):
                # previous pair's PV accumulation; row dh of each head's
                # block is the softmax denominator (ones column of v).
                st = {}

                def part(par, j):
                    if j == 0:
                        st[par] = psp.tile([P, SQC], F32, tag="acc",
                                           name=f"pv{ps_}_{2 * hp_ + par}")
                    pv = st[par]
                    _mm(nc, pv[0:DH + 1, :], v_sb[:, j, 2 * hp_ + par, :],
                        expb_[:, j, par, :], j == 0, j == NJ - 1)
                    if j == NJ - 1:
                        po = par * DH
                        nc.vector.tensor_copy(
                            attn_tiles[ps_][po:po + DH, hp_, :], pv[0:DH, :])
                        if par == 0:
                            # den rows live on partitions 0 and 32 (both
                            # 32-aligned starts) with zeros between, so one
                            # K=33 matmul broadcasts both heads' denominators
                            st["den"] = rc.tile([33, SQC], BF16, tag="den",
                                                name=f"dn{ps_}_{hp_}")
                            nc.vector.memset(st["den"][:], 0.0)
                        nc.vector.tensor_copy(
                            st["den"][32 * par:32 * par + 1, :],
                            pv[DH:DH + 1, :])

                def fin():
                    # broadcast den over each head's 64-partition block (one
                    # K=33 matmul; selm row 0 has ones on partitions 0..63,
                    # row 32 on 64..127), then reciprocal across all 128
                    # lanes and normalize that inner-chunk of attn
                    den = st["den"]
                    rps = psp.tile([P, SQC], F32, tag="acc",
                                   name=f"rb{ps_}_{hp_}")
                    _mm(nc, rps[:], selm_sb[:], den[:], True, True)
                    rrec = rc.tile([P, SQC], F32, tag="rrec",
                                   name=f"rr{ps_}_{hp_}")
                    nc.vector.reciprocal_approx_fast(rrec[:], rps[:])
                    nc.vector.tensor_mul(
                        attn_tiles[ps_][:, hp_, :],
                        attn_tiles[ps_][:, hp_, :], rrec[:])

                return part, fin

            def yo_group(s, r):
                # output projection + bias for one 128-row slice of chunk s
                y_sb = yp.tile([P, DQ], BF16, tag="y", name=f"y{s}_{r}")
                for nh in range(DQ // 512):
                    ps = psp.tile([P, 512], F32, tag="acc", name=f"yp{s}_{r}_{nh}")
                    for kc in range(NIC):
                        _mm(nc, ps[:],
                            attn_tiles[s][:, kc, r * P:(r + 1) * P],
                            Wo_sb[:, kc, nh * 512:(nh + 1) * 512],
                            kc == 0, kc == NIC - 1)
                    nc.vector.tensor_add(
                        y_sb[:, nh * 512:(nh + 1) * 512], ps[:],
                        bo_sb[:, nh * 512:(nh + 1) * 512])
                nc.sync.dma_start(
                    y[s * SQC + r * P: s * SQC + (r + 1) * P, :], y_sb[:])

            # --- HAM warm-up: the PE clock-gates to 1.2 GHz until it has
            # been busy ~3.4us. Burn the input-DMA wait on dummy matmuls
            # over a memset tile so the prefix and first body run at
            # 2.4 GHz instead of paying ~2x on ~14 matmuls.
            warm_sb = res.tile([P, 640], BF16)
            nc.vector.memset(warm_sb[:], 0.125)
            for w in range(12):
                wps = psp.tile([P, 512], F32, tag="acc", name=f"warm{w}")
                _mm(nc, wps[:], warm_sb[:, 0:P], warm_sb[:, P:640],
                    True, True)

            # --- minimal prefix: exactly what body (0,0) scores j0..j3
            # need (kT ic0/nk0 + qT s0/ic0); everything else is filler.
            # A second dummy block bridges the x0+Wq0 DMA wait so the HAM
            # stays warm for qT and the first body.
            kT_group(0, 0)
            for w in range(12, 24):
                wps = psp.tile([P, 512], F32, tag="acc", name=f"warm{w}")
                _mm(nc, wps[:], warm_sb[:, 0:P], warm_sb[:, P:640],
                    True, True)
            qT_group(0, 0)

            # --- filler machinery: (slot_weight, closure) queue, drained
            # in slot-weighted slices after each score of each body.
            fill = []

            def add(w, fn):
                fill.append((w, fn))

            def drain(target_w):
                done = 0.0
                while fill and done < target_w:
                    w, fn = fill.pop(0)
                    fn()
                    done += w

            def add_pv(part):
                for par in range(2):
                    for j in range(NJ):
                        add(1, lambda par=par, j=j: part(par, j))

            pend = None  # (part, fin) of the previous head pair

            # --- main pipeline over (query chunk, head pair) bodies ------
            for s in range(NSQ):
                attn_tiles[s] = at.tile([P, NIC, SQC], BF16, name=f"at{s}", tag="at")
                if s == 0:
                    pass
                elif s == 1:
                    load_xT(2)
                elif s == 2:
                    load_xT(3)
                for hp in range(NHP):
                    # build this body's filler list. fin goes LAST: with it
                    # early, the next three acc-ring allocations (qT, yo)
                    # all chain on fin's DVE latency (den copy -> rps ->
                    # reciprocal) and the PE stalls ~0.5-1us every body
                    # boundary.
                    if pend is not None:
                        part_prev, fin_prev = pend
                        add_pv(part_prev)
                        if (s, hp) == (NSQ - 1, NHP - 1):
                            # last body: fin mid-body so the epilogue's
                            # acc-ring allocations don't chain on its DVE
                            # latency at the critical tail.
                            add(1, fin_prev)
                            fin_prev = None
                    else:
                        fin_prev = None
                    if s == 0:
                        if hp == 0:
                            add(6, lambda: kT_group(0, 1))
                            add(6, lambda: kT_group(1, 0))
                            add(6, lambda: kT_group(1, 1))
                            for j in range(NJ):
                                add(6, lambda j=j: v_group(j))
                            add(8, lambda: qT_group(0, 1))
                        elif hp == 1:
                            add(6, lambda: kT_group(2, 0))
                            add(6, lambda: kT_group(2, 1))
                            add(8, lambda: qT_group(0, 2))
                        elif hp == 2:
                            add(6, lambda: kT_group(3, 0))
                            add(6, lambda: kT_group(3, 1))
                            add(8, lambda: qT_group(0, 3))
                            add(8, lambda: qT_group(1, 0))
                        else:
                            for ic in range(1, NIC):
                                add(8, lambda ic=ic: qT_group(1, ic))
                    else:
                        # yo(sp, r) must trail fin(sp, 3) (which finalizes
                        # attn[sp]) by at least one body, so the map is
                        # shifted: body (s,hp>=1) runs yo(s-1, hp-1) and
                        # body (s,0) runs yo(s-2, 3). yo(2,3) + yo(3,*)
                        # land in the epilogue.
                        if s + 1 < NSQ:
                            add(8, lambda s=s, hp=hp: qT_group(s + 1, hp))
                        if hp == 0:
                            if s >= 2:
                                add(8, lambda s=s: yo_group(s - 2, 3))
                        else:
                            add(8, lambda s=s, hp=hp: yo_group(s - 1, hp - 1))
                        if (s, hp) == (NSQ - 1, NHP - 1):
                            add(8, lambda: yo_group(2, 3))
                    if fin_prev is not None:
                        add(1, fin_prev)

                    total_w = sum(w for w, _ in fill)
                    qT_sb = qT_tiles[s]
                    expb = ex.tile([P, NJ, 2, SQC], BF16, tag="expb",
                                   name=f"eb{s}_{hp}")
                    emitted = 0.0
                    for j in range(NJ):
                        # scores^T [Skv, SQC] for the head pair; the two K=64
                        # matmuls land on disjoint PE row groups, share one
                        # 2-bank PSUM tile and one exp call, so the A/B row
                        # groups execute concurrently. Scores are emitted in
                        # j-PAIRS (the 3-deep sc ring gives the second score
                        # the slack) so the filler blocks between them stay
                        # large — every group boundary in the PE stream costs
                        # ~100ns of exposed LDWEIGHTS.
                        sps = psp.tile([P, 2, 512], F32, tag="sc", bufs=3,
                                       name=f"sc{s}_{hp}_{j}")
                        _mm(nc, sps[:, 0, :],
                            kT_sb[0:DH, hp, j * P:(j + 1) * P],
                            qT_sb[0:DH, hp, :], True, True)
                        _mm(nc, sps[:, 1, :],
                            kT_sb[DH:P, hp, j * P:(j + 1) * P],
                            qT_sb[DH:P, hp, :], True, True)
                        nc.scalar.activation(
                            expb[:, j, :, :], sps[:], EXP, scale=SCALE)
                        if j % 2 == 1:
                            quota = total_w * (j + 1) / NJ - emitted
                            drain(quota)
                            emitted += quota
                    pend = make_pv(s, hp, expb)

            # --- epilogue: drain leftovers, last pair's PV + fin. The last
            # chunk's output projection is split partial-K: the ic0..2
            # contributions (normalized several bodies ago) pre-accumulate
            # into the score-ring banks (free once the last exps retire)
            # while fin's DVE chain runs, and only the 8 ic3 finishing
            # matmuls remain on the post-fin critical path.
            drain(1e9)
            part_last, fin_last = pend
            for par in range(2):
                for j in range(NJ):
                    part_last(par, j)
            fin_last()
            sl = NSQ - 1
            ypp = {}
            for r in range(3):
                t = psp.tile([P, 2, 512], F32, tag="sc", bufs=3,
                             name=f"ypp{r}")
                for nh in range(2):
                    ypp[(r, nh)] = t[:, nh, :]
                    for kc in range(NIC - 1):
                        _mm(nc, t[:, nh, :],
                            attn_tiles[sl][:, kc, r * P:(r + 1) * P],
                            Wo_sb[:, kc, nh * 512:(nh + 1) * 512],
                            kc == 0, False)
            for nh in range(2):
                t = psp.tile([P, 512], F32, tag="acc", name=f"ypp3_{nh}")
                ypp[(3, nh)] = t[:]
                for kc in range(NIC - 1):
                    _mm(nc, t[:],
                        attn_tiles[sl][:, kc, 3 * P:4 * P],
                        Wo_sb[:, kc, nh * 512:(nh + 1) * 512],
                        kc == 0, False)
            for r in range(NHP):
                # 4-deep ring: with bufs=2 the third slice's bias-add waits
                # the first slice's output DMA and the tail serializes.
                y_sb = yp.tile([P, DQ], BF16, tag="yf", bufs=4, name=f"yf{r}")
                for nh in range(2):
                    _mm(nc, ypp[(r, nh)],
                        attn_tiles[sl][:, NIC - 1, r * P:(r + 1) * P],
                        Wo_sb[:, NIC - 1, nh * 512:(nh + 1) * 512],
                        False, True)
                    nc.vector.tensor_add(
                        y_sb[:, nh * 512:(nh + 1) * 512], ypp[(r, nh)],
                        bo_sb[:, nh * 512:(nh + 1) * 512])
                nc.sync.dma_start(
                    y[sl * SQC + r * P: sl * SQC + (r + 1) * P, :], y_sb[:])
    nc.compile()
    return nc


_NC_CACHE = None


def kernel(x, context, Wq, Wk, Wv, Wo, bo):
    global _NC_CACHE
    import ml_dtypes
    bf16 = ml_dtypes.bfloat16

    x = np.asarray(x, dtype=np.float32)
    context = np.asarray(context, dtype=np.float32)
    # Wq/Wk packed ic-major: [ic][p][kc][n] with element (ic,p,kc,n) =
    # W[kc*128+p, ic*128+n], so the kernel's prefix can DMA just the ic0
    # slice contiguously.
    Wq_b = np.asarray(Wq, dtype=np.float32).astype(bf16) \
        .reshape(KCQ, P, NIC, P).transpose(2, 1, 0, 3)
    Wq_b = np.ascontiguousarray(Wq_b)
    Wk_b = np.asarray(Wk, dtype=np.float32).astype(bf16) \
        .reshape(KCK, P, NIC, P).transpose(2, 1, 0, 3)
    Wk_b = np.ascontiguousarray(Wk_b)
    Wv_b = np.ascontiguousarray(np.asarray(Wv, dtype=np.float32).astype(bf16))
    Wo_b = np.ascontiguousarray(np.asarray(Wo, dtype=np.float32).astype(bf16))
    bo = np.ascontiguousarray(np.asarray(bo, dtype=np.float32))

    if _NC_CACHE is None:
        _NC_CACHE = build_nc()
    nc = _NC_CACHE

    selm = np.zeros((33, P), dtype=np.float32)
    selm[0, 0:DH] = 1.0
    selm[32, DH:2 * DH] = 1.0
    selm = np.ascontiguousarray(selm.astype(bf16))

    in_maps = []
    for c in range(8):
        b, half = c // 2, c % 2
        xs = x[b, half * SQ:(half + 1) * SQ, :]            # [2048, 1024]
        in_maps.append({
            "xT": np.ascontiguousarray(xs.T.astype(bf16)),       # [1024, 2048]
            "ctxT": np.ascontiguousarray(context[b].T.astype(bf16)),  # [768, 1024]
            "Wq": Wq_b, "Wk": Wk_b, "Wv": Wv_b, "Wo": Wo_b, "bo": bo,
            "selm": selm,
        })

    trace = bool(int(os.environ.get("KERNEL_TRACE", "0")))
    res = run_bass_kernel_spmd(nc, in_maps, core_ids=list(range(8)), trace=trace)
    kernel.last_results = res

    out = np.empty((B, SQ_FULL, DQ), dtype=np.float32)
    for c in range(8):
        b, half = c // 2, c % 2
        out[b, half * SQ:(half + 1) * SQ, :] = \
            res.results[c]["y"].astype(np.float32)
    return out
